# Initial kernel scaffold
#
"""BlockwiseQuantLinear on 8 trn2 NeuronCores.

y = act_quant_dequant(x) @ (fp8_weight * block_scales).T
  x: [8192, 2048] f32, weight: [2048, 2048] fp8_e4m3fn (OCP), w_scale: [16, 16] f32
  out: [8192, 2048] f32

Strategy (data-parallel over tokens; hardcoded shapes):
  - Host: dequantize the static weight to bf16 (exact wrt reference up to bf16
    rounding) and lay it out K-major in 512-wide N chunks for direct DMA of
    [k_inner=128, k_block, n] SBUF tiles. Shard x rows 8 ways.
  - Device (per core, M_sh=1024): for each 128-row tile of x:
      blockwise act quant: amax over each (1,128) k-block -> scale; multiply by
      224/amax and cast to TRN fp8e4 (numerically == OCP e4m3fn quant at half
      scale; TRN max normal is 240 so the half grid keeps values <= 224),
      dequantize to bf16 with amax/224, DMA-transpose to [k, m] layout, then a
      single PSUM-accumulated bf16 GEMM over all 16 k-blocks (scales are fully
      folded into the operands, so no per-block rescale is needed).
  - Gather: concatenate the 8 row shards.
"""

import numpy as np
import ml_dtypes

import concourse.bass as bass
import concourse.mybir as mybir
import concourse.tile as tile
from concourse import bacc
from concourse.bass_utils import run_bass_kernel_spmd

P = 128
M, K, N = 8192, 2048, 2048
NCORES = 8
M_SH = M // NCORES            # 1024 rows per core
MT = M_SH // P                # 8 m-tiles per core
KB = K // P                   # 16 k blocks
NCH = 4                       # n chunks of 512
NC_W = N // NCH               # 512
EPS = 1e-12

_cache = {}


def _build():
    nc = bacc.Bacc(None, target_bir_lowering=False)

    x_in = nc.dram_tensor("x_sh", [M_SH, K], mybir.dt.float32, kind="ExternalInput")
    # [nch, k_inner, k_block, n_in_chunk], fully contiguous per DMA chunk
    w_in = nc.dram_tensor("wT", [NCH, P, KB, NC_W], mybir.dt.bfloat16, kind="ExternalInput")
    y_out = nc.dram_tensor("y_sh", [M_SH, N], mybir.dt.float32, kind="ExternalOutput")

    with tile.TileContext(nc) as tc:
        with (
            tc.tile_pool(name="wpool", bufs=1) as wpool,
            tc.tile_pool(name="xpool", bufs=3) as xpool,
            tc.tile_pool(name="qpool", bufs=3) as qpool,
            tc.tile_pool(name="spool", bufs=3) as spool,
            tc.tile_pool(name="ypool", bufs=2) as ypool,
            tc.tile_pool(name="ps", bufs=2, space="PSUM") as ps,
        ):
            wts = []
            for nch in range(NCH):
                wt = wpool.tile([P, KB, NC_W], mybir.dt.bfloat16, name=f"w{nch}")
                nc.sync.dma_start(wt[:], w_in[nch])
                wts.append(wt)

            for mi in range(MT):
                x_sb = xpool.tile([P, K], mybir.dt.float32, name="x_sb")
                nc.sync.dma_start(x_sb[:], x_in[bass.ts(mi, P), :])
                x3 = x_sb[:].rearrange("p (kb ki) -> p kb ki", kb=KB)

                amax = spool.tile([P, KB], mybir.dt.float32, name="amax")
                nc.vector.tensor_reduce(
                    amax[:], x3, axis=mybir.AxisListType.X,
                    op=mybir.AluOpType.max, apply_absolute_value=True,
                )
                amaxp = spool.tile([P, KB], mybir.dt.float32, name="amaxp")
                nc.vector.tensor_scalar_max(amaxp[:], amax[:], EPS)
                rec = spool.tile([P, KB], mybir.dt.float32, name="rec")
                nc.vector.reciprocal(rec[:], amaxp[:])
                inv2 = spool.tile([P, KB], mybir.dt.float32, name="inv2")
                nc.vector.tensor_scalar_mul(inv2[:], rec[:], 224.0)
                s2 = spool.tile([P, KB], mybir.dt.float32, name="s2")
                nc.vector.tensor_scalar_mul(s2[:], amaxp[:], 1.0 / 224.0)

                t8 = qpool.tile([P, K], mybir.dt.float8e4, name="t8")
                t83 = t8[:].rearrange("p (kb ki) -> p kb ki", kb=KB)
                nc.vector.tensor_tensor(
                    t83, x3, inv2[:, :, None].to_broadcast([P, KB, P]),
                    mybir.AluOpType.mult,
                )
                xdq = qpool.tile([P, K], mybir.dt.bfloat16, name="xdq")
                xdq3 = xdq[:].rearrange("p (kb ki) -> p kb ki", kb=KB)
                nc.vector.tensor_tensor(
                    xdq3, t83, s2[:, :, None].to_broadcast([P, KB, P]),
                    mybir.AluOpType.mult,
                )

                xT = qpool.tile([P, KB, P], mybir.dt.bfloat16, name="xT")
                nc.sync.dma_start_transpose(xT[:], xdq[:])

                y_sb = ypool.tile([P, N], mybir.dt.float32, name="y_sb")
                for nch in range(NCH):
                    psum = ps.tile([P, NC_W], mybir.dt.float32, name=f"ps{nch}")
                    for kb in range(KB):
                        nc.tensor.matmul(
                            psum[:], xT[:, kb, :], wts[nch][:, kb, :],
                            start=(kb == 0), stop=(kb == KB - 1),
                        )
                    nc.vector.tensor_copy(y_sb[:, bass.ts(nch, NC_W)], psum[:])
                nc.sync.dma_start(y_out[bass.ts(mi, P), :], y_sb[:])

    nc.compile()
    return nc


def _prep_weight(weight: np.ndarray, w_scale: np.ndarray) -> np.ndarray:
    w_f32 = weight.astype(np.float32)                     # exact
    ws_full = np.repeat(np.repeat(w_scale.astype(np.float32), P, axis=0), P, axis=1)
    w_deq = (w_f32 * ws_full).astype(ml_dtypes.bfloat16)  # [N, K]
    # -> [nch, ki, kb, n_in_chunk]: element [c, ki, kb, n] = w_deq[c*512 + n, kb*128 + ki]
    wt = np.ascontiguousarray(
        w_deq.T.reshape(KB, P, NCH, NC_W).transpose(2, 1, 0, 3)
    )
    return wt


def kernel(x: np.ndarray, weight: np.ndarray, w_scale: np.ndarray, _trace: bool = False):
    if "nc" not in _cache:
        _cache["nc"] = _build()
    nc = _cache["nc"]

    wt = _prep_weight(weight, w_scale)
    x = np.ascontiguousarray(x, dtype=np.float32)

    in_maps = [
        {"x_sh": x[c * M_SH:(c + 1) * M_SH], "wT": wt}
        for c in range(NCORES)
    ]
    res = run_bass_kernel_spmd(
        nc, in_maps, core_ids=list(range(NCORES)),
        trace=_trace, trace_cores=list(range(NCORES)) if _trace else None,
    )
    y = np.concatenate([res.results[c]["y_sh"] for c in range(NCORES)], axis=0)
    if _trace:
        kernel.last_results = res
    return y


# revision 1
# speedup vs baseline: 1.2256x; 1.2256x over previous
"""BlockwiseQuantLinear on 8 trn2 NeuronCores.

y = act_quant_dequant(x) @ (fp8_weight * block_scales).T
  x: [8192, 2048] f32, weight: [2048, 2048] fp8_e4m3fn (OCP), w_scale: [16, 16] f32
  out: [8192, 2048] f32

Strategy (data-parallel over tokens; hardcoded shapes):
  - Host: dequantize the static weight to bf16 (exact wrt reference up to bf16
    rounding) and lay it out K-major in 512-wide N chunks for direct DMA of
    [k_inner=128, k_block, n] SBUF tiles. Shard x rows 8 ways.
  - Device (per core, M_sh=1024): for each 128-row tile of x:
      blockwise act quant: amax over each (1,128) k-block -> scale; multiply by
      224/amax and cast to TRN fp8e4 (numerically == OCP e4m3fn quant at half
      scale; TRN max normal is 240 so the half grid keeps values <= 224),
      dequantize to bf16 with amax/224, DMA-transpose to [k, m] layout, then a
      single PSUM-accumulated bf16 GEMM over all 16 k-blocks (scales are fully
      folded into the operands, so no per-block rescale is needed).
  - Gather: concatenate the 8 row shards.
"""

import numpy as np
import ml_dtypes

import concourse.bass as bass
import concourse.mybir as mybir
import concourse.tile as tile
from concourse import bacc
from concourse.bass_utils import run_bass_kernel_spmd

P = 128
M, K, N = 8192, 2048, 2048
NCORES = 8
M_SH = M // NCORES            # 1024 rows per core
MT = M_SH // P                # 8 m-tiles per core
KB = K // P                   # 16 k blocks
NCH = 4                       # n chunks of 512
NC_W = N // NCH               # 512
EPS = 1e-12

_cache = {}


def _build():
    nc = bacc.Bacc(None, target_bir_lowering=False)

    x_in = nc.dram_tensor("x_sh", [M_SH, K], mybir.dt.float32, kind="ExternalInput")
    # [nch, k_inner, k_block, n_in_chunk], fully contiguous per DMA chunk
    w_in = nc.dram_tensor("wT", [NCH, P, KB, NC_W], mybir.dt.bfloat16, kind="ExternalInput")
    y_out = nc.dram_tensor("y_sh", [M_SH, N], mybir.dt.float32, kind="ExternalOutput")

    with tile.TileContext(nc) as tc:
        with (
            tc.tile_pool(name="wpool", bufs=1) as wpool,
            tc.tile_pool(name="xpool", bufs=3) as xpool,
            tc.tile_pool(name="qpool", bufs=3) as qpool,
            tc.tile_pool(name="spool", bufs=3) as spool,
            tc.tile_pool(name="ypool", bufs=2) as ypool,
            tc.tile_pool(name="ps", bufs=2, space="PSUM") as ps,
        ):
            wts = []
            for nch in range(NCH):
                wt = wpool.tile([P, KB, NC_W], mybir.dt.bfloat16, name=f"w{nch}")
                nc.sync.dma_start(wt[:], w_in[nch])
                wts.append(wt)

            for mi in range(MT):
                x_sb = xpool.tile([P, K], mybir.dt.float32, name="x_sb")
                nc.sync.dma_start(x_sb[:], x_in[bass.ts(mi, P), :])
                x3 = x_sb[:].rearrange("p (kb ki) -> p kb ki", kb=KB)

                amax = spool.tile([P, KB], mybir.dt.float32, name="amax")
                nc.vector.tensor_reduce(
                    amax[:], x3, axis=mybir.AxisListType.X,
                    op=mybir.AluOpType.max, apply_absolute_value=True,
                )
                amaxp = spool.tile([P, KB], mybir.dt.float32, name="amaxp")
                nc.vector.tensor_scalar_max(amaxp[:], amax[:], EPS)
                rec = spool.tile([P, KB], mybir.dt.float32, name="rec")
                nc.vector.reciprocal(rec[:], amaxp[:])
                inv2 = spool.tile([P, KB], mybir.dt.float32, name="inv2")
                nc.vector.tensor_scalar_mul(inv2[:], rec[:], 224.0)
                s2 = spool.tile([P, KB], mybir.dt.float32, name="s2")
                nc.vector.tensor_scalar_mul(s2[:], amaxp[:], 1.0 / 224.0)

                t8 = qpool.tile([P, K], mybir.dt.float8e4, name="t8")
                t83 = t8[:].rearrange("p (kb ki) -> p kb ki", kb=KB)
                nc.vector.tensor_tensor(
                    t83, x3, inv2[:, :, None].to_broadcast([P, KB, P]),
                    mybir.AluOpType.mult,
                )
                xdq = qpool.tile([P, K], mybir.dt.bfloat16, name="xdq")
                xdq3 = xdq[:].rearrange("p (kb ki) -> p kb ki", kb=KB)
                nc.vector.tensor_tensor(
                    xdq3, t83, s2[:, :, None].to_broadcast([P, KB, P]),
                    mybir.AluOpType.mult,
                )

                xT = qpool.tile([P, KB, P], mybir.dt.bfloat16, name="xT")
                nc.sync.dma_start_transpose(xT[:], xdq[:])

                y_sb = ypool.tile([P, N], mybir.dt.float32, name="y_sb")
                for nch in range(NCH):
                    psum = ps.tile([P, NC_W], mybir.dt.float32, name=f"ps{nch}")
                    for kb in range(KB):
                        nc.tensor.matmul(
                            psum[:], xT[:, kb, :], wts[nch][:, kb, :],
                            start=(kb == 0), stop=(kb == KB - 1),
                        )
                    nc.vector.tensor_copy(y_sb[:, bass.ts(nch, NC_W)], psum[:])
                nc.sync.dma_start(y_out[bass.ts(mi, P), :], y_sb[:])

    nc.compile()
    return nc


def _prep_weight(weight: np.ndarray, w_scale: np.ndarray) -> np.ndarray:
    w_f32 = weight.astype(np.float32)                     # exact
    ws_full = np.repeat(np.repeat(w_scale.astype(np.float32), P, axis=0), P, axis=1)
    w_deq = (w_f32 * ws_full).astype(ml_dtypes.bfloat16)  # [N, K]
    # -> [nch, ki, kb, n_in_chunk]: element [c, ki, kb, n] = w_deq[c*512 + n, kb*128 + ki]
    wt = np.ascontiguousarray(
        w_deq.T.reshape(KB, P, NCH, NC_W).transpose(2, 1, 0, 3)
    )
    return wt


def kernel(x: np.ndarray, weight: np.ndarray, w_scale: np.ndarray, _trace: bool = False):
    if "nc" not in _cache:
        _cache["nc"] = _build()
    nc = _cache["nc"]

    wt = _prep_weight(weight, w_scale)
    x = np.ascontiguousarray(x, dtype=np.float32)

    in_maps = [
        {"x_sh": x[c * M_SH:(c + 1) * M_SH], "wT": wt}
        for c in range(NCORES)
    ]
    res = run_bass_kernel_spmd(
        nc, in_maps, core_ids=list(range(NCORES)),
        trace=_trace, trace_cores=list(range(NCORES)) if _trace else None,
    )
    y = np.concatenate([res.results[c]["y_sh"] for c in range(NCORES)], axis=0)
    if _trace:
        kernel.last_results = res
    return y
